# revision 31
# baseline (speedup 1.0000x reference)
"""Trainium2 Bass kernel for OneSideInterModalityUpdate (dense transformer block).

Reference computation (per batch b, one NeuronCore each -- data-parallel B=8):
    src_tran = relu(src @ W_src)                  [Ns, 2*OUT]
    key, val = split(src_tran)                    [Ns, OUT] each
    q        = relu(tgt @ W_tgt)                  [Nt, OUT]
    per head h (12 heads, DH=64):
        S     = q_h @ k_h^T / sqrt(DH)            [Nt, Ns]
        A     = softmax(S, axis=-1)
        upd_h = A @ v_h                           [Nt, DH]
    out = relu([tgt, upd] @ W_out)                [Nt, OUT]

v2 performance design (v1 measured: PE 175us busy of a 218us span, ACT
160us busy -- dual-bottleneck.  v2 attacks both):

  - K/Q/V projections run fp8e4 DoubleRow (src/tgt/W_src/W_tgt shipped
    as fp8): contraction 768 = 3 dual-128 k-tile instructions instead of
    6, halving projection PE streaming (fp8 dot noise is ~5% per K/Q/V
    element but attention-averages out over ~800 source positions;
    numpy-validated end-to-end at 2.8e-3 vs 2.5e-3 for bf16).  Scores,
    AV (fp8 es/v65, DoubleRow) and the out-projection (bf16) keep v1's
    layouts and numerics.
  - The exp stream -- v1's 129us serial ACT chain -- is split across TWO
    engines: tb=0 scores tiles exp on ACT (exact, scale/bias folded);
    tb=1 tiles are computed on the DVE as a ONE-instruction Schraudolph:
    fp8e4 bits of exp(x) are round(x*8/ln2 + 56), i.e. one
    tensor_scalar(mult,add) f32->int8 written through an int8 bitcast of
    the fp8 es tile.  The fp8 grid (ulp = 2^(1/8)) swallows the
    mantissa-linearization error, so this is exactly as accurate as
    exact-exp-then-cast-to-fp8; a whole (t,head) softmax row stays on
    one engine so any systematic rounding bias cancels in Z.
  - V gets a ones-column (65th lhsT column) so AV psum row 64 is the
    softmax denominator Z for free; Z -> DRAM -> [128,16] repack ->
    reciprocal -> broadcast-back normalization as in v1.
  - Output projection TRANSPOSED (wout stationary), prefix/park/close
    split so only the pair-5 chunk trails the last exp.  Host transposes
    the [OUT, NT] bf16 result back.
  - Startup: fp8 halves the activation bytes; DMAs are staged
    first-need-first (K/Q column-block 0 + first src/tgt chunk pairs
    lead both HWDGE queues) so the first projection matmuls start ~3us
    in and the first exp ~7us (v1: 22us).
"""

import numpy as np
import ml_dtypes

import concourse.bass as bass
import concourse.mybir as mybir
import concourse.tile as tile
from concourse import library_config
from concourse.bass_utils import run_bass_kernel_spmd

BF16 = mybir.dt.bfloat16
F32 = mybir.dt.float32
FP8 = mybir.dt.float8e4
I8 = mybir.dt.int8
AF = mybir.ActivationFunctionType
ALU = mybir.AluOpType
PM = mybir.MatmulPerfMode

B, NS, NT = 8, 1024, 1024
SRC, TGT, OUT, H = 768, 768, 768, 12
_IDENT = np.eye(128, dtype=ml_dtypes.bfloat16)
_FP8 = ml_dtypes.float8_e4m3
DH = OUT // H            # 64
P = 128
NKC = SRC // P           # 6 contraction chunks for the projections
NJ = NKC // 2            # 3 dual-k-tile DoubleRow steps
NSC = NS // P            # 8 source chunks
NG = H // 2              # 6 head pairs
NU = 12                  # out-proj units: 6 o-chunks x 2 t-halves
SCALE = 1.0 / np.sqrt(DH)
# Schraudolph exp->fp8e4 bits: bits = round((s*SCALE - 1) * 8/ln2 + 56)
_L2E8 = 8.0 / np.log(2.0)
EXPA = float(SCALE * _L2E8)
EXPB = float(56.0 - _L2E8)

_NC_CACHE = None


def _split_excess_waits(nc, keep=1):
    """This container's walrus encodes at most ONE sync-wait per instruction,
    but the Tile scheduler can attach several (notably on the final drain).
    Split excess waits onto preceding same-engine NoOp carriers."""
    for fn in nc.m.functions:
        for bb in fn.blocks:
            il = list(bb.instructions)
            out = []
            changed = False
            for inst in il:
                si = inst.sync_info
                if si is not None and len(si.on_wait) > keep:
                    waits = list(si.on_wait)
                    changed = True
                    ncarry = len(waits) - keep
                    for i0 in range(0, ncarry, keep):
                        nop = mybir.InstNoOp(
                            name=nc.get_next_instruction_name(),
                            opcode="NoOp",
                            engine=inst.engine,
                            debug=inst.debug,
                            ins=[],
                            outs=[],
                            descendants=None,
                            sync_info=mybir.SyncInfo(
                                on_wait=waits[i0 : i0 + keep], on_update=[]
                            ),
                            bass_sim_breakpoint=False,
                            bass_priority=None,
                            bass_wait_until_ts=None,
                            bass_scheduled_tick=None,
                            bass_scheduled_proc=None,
                            bass_scheduled_scope=None,
                            bass_addl_debug=None,
                            text_hint="wait_carrier",
                            bass_nofuse=True,
                        )
                        nc.register_instruction(nop)
                        out.append(nop)
                    inst.sync_info = mybir.SyncInfo(
                        on_wait=waits[ncarry:], on_update=list(si.on_update)
                    )
                out.append(inst)
            if changed:
                bb.instructions = out
    return nc


def _build_nc() -> bass.Bass:
    nc = bass.Bass()

    # Each HWDGE queue's FIRST transfer is the whole critical startup set
    # for its side, packed contiguously: every dma_start pays ~2us of fixed
    # completion latency that serializes on its queue, so two 4-piece queues
    # put the first matmul at ~13.5us -- one blob per queue puts it at ~12.5
    # with the full set landed.  blob = [W column-block 0 | all 6 activation
    # chunk-pairs] per partition.
    BLOB = NKC * P + NKC * NS  # 6912 fp8 bytes per partition
    bsrc_d = nc.dram_tensor("bsrc", [P, BLOB], FP8, kind="ExternalInput")
    btgt_d = nc.dram_tensor("btgt", [P, BLOB], FP8, kind="ExternalInput")
    # remaining W pieces ship as separate contiguous tensors: a column-slice
    # DMA out of one big [P, k, 1536] tensor moves 128-640B elements and
    # crawls at ~60GB/s on the wire (measured: the K-rest slice completed at
    # 31us and stalled the PE 9us); per-piece tensors stream at full rate.
    wkrest_d = nc.dram_tensor("wkrest", [P, NKC, OUT - P], FP8, kind="ExternalInput")
    wval_d = nc.dram_tensor("wval", [P, NKC, OUT], FP8, kind="ExternalInput")
    wqrest_d = nc.dram_tensor("wqrest", [P, NKC, OUT - P], FP8, kind="ExternalInput")
    tgtT_d = nc.dram_tensor("tgtT", [TGT, NT], BF16, kind="ExternalInput")
    wout_d = nc.dram_tensor("w_out", [OUT + TGT, OUT], BF16, kind="ExternalInput")
    outT_d = nc.dram_tensor("out", [OUT, NT], BF16, kind="ExternalOutput")
    ident_d = nc.dram_tensor("ident", [P, P], BF16, kind="ExternalInput")

    with tile.TileContext(nc) as tc:
        with (
            tc.tile_pool(name="const", bufs=1) as cpool,
            tc.tile_pool(name="es", bufs=2) as epool,
            tc.tile_pool(name="rr", bufs=2) as rpool,
            tc.tile_pool(name="outsb", bufs=3) as opool,
            tc.tile_pool(name="pss", bufs=2, space="PSUM") as pss,
            tc.tile_pool(name="pav", bufs=4, space="PSUM") as pav,
            tc.tile_pool(name="dram", bufs=2, space="DRAM") as dpool,
        ):
            # ---- persistent SBUF tensors ----
            bsrc = cpool.tile([P, BLOB], FP8)
            btgt = cpool.tile([P, BLOB], FP8)
            wk0 = bsrc[:, 0 : NKC * P].rearrange("p (k c) -> p k c", k=NKC)
            src8 = bsrc[:, NKC * P :].rearrange("p (k n) -> p k n", k=NKC)
            wq0 = btgt[:, 0 : NKC * P].rearrange("p (k c) -> p k c", k=NKC)
            tgt8 = btgt[:, NKC * P :].rearrange("p (k n) -> p k n", k=NKC)
            wkrest = cpool.tile([P, NKC, OUT - P], FP8)
            wval = cpool.tile([P, NKC, OUT], FP8)
            wqrest = cpool.tile([P, NKC, OUT - P], FP8)
            tgtTk = [cpool.tile([P, NT], BF16, name=f"tgtc{j}") for j in range(NKC)]
            wout = cpool.tile([P, 2 * NKC, OUT], BF16)
            kTh = [cpool.tile([P, NS], BF16, name=f"kT{g}") for g in range(NG)]
            qTh = [cpool.tile([P, NT], BF16, name=f"qT{g}") for g in range(NG)]
            # [s, h, sc, dh+ones+pad]: the sc slot is padded to 80 bytes because
            # DoubleRow LDWEIGHTS requires the dual-k-tile stride %16 == 0.
            v65 = cpool.tile([P, H, NSC, 80], FP8)
            updk = [cpool.tile([P, NT], BF16, name=f"upd{g}") for g in range(NG)]
            part_u = [cpool.tile([P, 512], BF16, name=f"pout{u}") for u in range(NU)]
            ident = cpool.tile([P, P], BF16)
            neg1 = cpool.tile([P, 1], F32)
            prime = cpool.tile([1, 1], F32)

            # ---- input DMAs.  The two HWDGE queues carry everything the
            # stream needs, strictly in consumption order -- queue order
            # serializes the transfers, so the non-critical pieces cannot
            # steal HBM bandwidth from the first-needed ones.  The bulky
            # bf16 late-need tensors (tgtT chunks for the out-proj moving
            # side, wout, ident; needed only from block 4, ~80us in) ride
            # SWDGE on the Pool engine, which is GATED below so its HBM
            # pulls don't overlap the critical startup set either. ----
            nc.sync.dma_start(bsrc[:], bsrc_d[:])
            nc.scalar.dma_start(btgt[:], btgt_d[:])
            nc.sync.dma_start(wkrest[:], wkrest_d[:])
            nc.scalar.dma_start(wqrest[:], wqrest_d[:])
            nc.sync.dma_start(wval[:], wval_d[:])
            gate = cpool.tile([1, 1], BF16)

            nc.vector.memset(v65[:, :, :, DH], 1.0)  # ones column for Z
            nc.vector.memset(neg1[:], -1.0)  # exp bias (cancels in softmax)
            # prime the ACT exp table NOW -- the implicit load otherwise
            # rides along with the first real exp's dispatch (1.3us late)
            nc.scalar.activation(prime[:], neg1[0:1, :], AF.Exp)

            # ---- building blocks ----
            nev = [0]

            def kq_chunk(which, mo, tb):
                # one [128,512] half of a K^T/Q^T projection column block:
                # 3 accumulating fp8 DoubleRow matmuls + relu evict
                # (alternating DVE/ACT -- both carry half the exp stream).
                w0, wrest, act8, dst = (
                    (wk0, wkrest, src8, kTh)
                    if which == 0
                    else (wq0, wqrest, tgt8, qTh)
                )
                ps = pav.tile([P, 512], F32, tag="pav", name=f"kq{which}_{mo}_{tb}")
                sl = slice(tb * 512, (tb + 1) * 512)
                for j in range(NJ):
                    w = (
                        w0[:, 2 * j : 2 * j + 2, :]
                        if mo == 0
                        else wrest[:, 2 * j : 2 * j + 2, (mo - 1) * P : mo * P]
                    )
                    nc.tensor.matmul(
                        ps[:],
                        w,
                        act8[:, 2 * j : 2 * j + 2, sl],
                        start=(j == 0),
                        stop=(j == NJ - 1),
                        perf_mode=PM.DoubleRow,
                    )
                nev[0] += 1
                if nev[0] % 2 == 0:
                    nc.vector.tensor_scalar_max(dst[mo][:, sl], ps[:], 0.0)
                else:
                    nc.scalar.activation(dst[mo][:, sl], ps[:], AF.Relu)

            es_tiles = {}

            def scores_open(g):
                es_tiles[g] = (
                    epool.tile([P, 2, NSC, 512], FP8, tag="esA", name=f"esA{g}"),
                    epool.tile([P, 2, NSC, 512], FP8, tag="esB", name=f"esB{g}"),
                )

            def scores_sc(g, sc):
                esA, esB = es_tiles[g]
                for tb, es in ((0, esA), (1, esB)):
                    ps = pss.tile([P, 2, 512], F32, tag="pss", name=f"sc{g}_{sc}")
                    for h01 in range(2):
                        hp = h01 * DH
                        nc.tensor.matmul(
                            ps[:, h01, :],
                            kTh[g][hp : hp + DH, sc * P : (sc + 1) * P],
                            qTh[g][hp : hp + DH, tb * 512 : (tb + 1) * 512],
                            start=True,
                            stop=True,
                        )
                    # the psum-ring slot for tb is freed by whichever engine
                    # exps this tile; alternating the assignment per sc means
                    # consecutive frees of a slot come from BOTH engines, so
                    # one engine running momentarily behind doesn't stall the
                    # next scores matmul (measured ~0.7us stalls otherwise).
                    if tb == (sc % 2):
                        # bias -1 (cancels in softmax) keeps es in [0.4, 110]:
                        # comfortably inside fp8e4's finite range, far above
                        # its subnormal floor.
                        nc.scalar.activation(
                            es[:, :, sc], ps[:], AF.Exp, bias=neg1[:], scale=SCALE
                        )
                    else:
                        # Schraudolph: fp8 bits of exp(s*SCALE-1) in ONE DVE op
                        nc.vector.tensor_scalar(
                            es[:, :, sc].bitcast(I8),
                            ps[:],
                            EXPA,
                            EXPB,
                            ALU.mult,
                            ALU.add,
                        )

            def v_chunk(sc, vh):
                # V columns for head-half vh (6 heads), one source chunk,
                # via 3 DoubleRow matmuls (src chunk stationary, W_val moving)
                o0 = vh * 384
                ps = pav.tile([P, 384], F32, tag="pav", name=f"vp{vh}_{sc}")
                for j in range(NJ):
                    nc.tensor.matmul(
                        ps[:, :],
                        src8[:, 2 * j : 2 * j + 2, sc * P : (sc + 1) * P],
                        wval[:, 2 * j : 2 * j + 2, o0 : o0 + 384],
                        start=(j == 0),
                        stop=(j == NJ - 1),
                        perf_mode=PM.DoubleRow,
                    )
                nev[0] += 1
                dst = v65[:, 6 * vh : 6 * (vh + 1), sc, 0:DH]
                srcv = ps[:].rearrange("p (h c) -> p h c", c=DH)
                if nev[0] % 2 == 0:
                    nc.vector.tensor_scalar_max(dst, srcv, 0.0)
                else:
                    nc.scalar.activation(dst, srcv, AF.Relu)

            pu_live = {}

            def av_open(g):
                pu_live[g] = [
                    [
                        pav.tile([P, 512], F32, tag="pav", name=f"pu{g}_{h01}_{tb}")
                        for tb in range(2)
                    ]
                    for h01 in range(2)
                ]

            def av_chunk(g, p4):
                esA, esB = es_tiles[g]
                pu = pu_live[g]
                for h01 in range(2):
                    h = 2 * g + h01
                    for tb, es in ((0, esA), (1, esB)):
                        nc.tensor.matmul(
                            pu[h01][tb][0 : DH + 1, :],
                            v65[:, h, 2 * p4 : 2 * p4 + 2, 0 : DH + 1],
                            es[:, h01, 2 * p4 : 2 * p4 + 2, :],
                            start=(p4 == 0),
                            stop=(p4 == NSC // 2 - 1),
                            perf_mode=PM.DoubleRow,
                        )

            def norm(g):
                # evict rows 0..64 of each psum quarter into one staging
                # mega-tile (frees the pav slots fast -- the ring is shared
                # with the projection/out-proj chunks), then: Z row -> DRAM
                # -> [128,16] repack -> cheap reciprocal -> DRAM -> broadcast
                # R -> normalize into updk.
                es_tiles.pop(g)
                pu = pu_live.pop(g)
                stg = rpool.tile([P, 4, 512], F32, tag="stg", name=f"stg{g}")
                for h01 in range(2):
                    for tb in range(2):
                        # last pair: the exp stream is over and this chain
                        # gates the final closes -- split across both engines
                        eng = (
                            (nc.scalar if tb == 0 else nc.vector)
                            if g == NG - 1
                            else nc.scalar
                        )
                        if eng is nc.vector:
                            eng.tensor_copy(
                                stg[0 : DH + 1, 2 * h01 + tb, :],
                                pu[h01][tb][0 : DH + 1, :],
                            )
                        else:
                            eng.copy(
                                stg[0 : DH + 1, 2 * h01 + tb, :],
                                pu[h01][tb][0 : DH + 1, :],
                            )
                z_dram = dpool.tile([1, 2 * NT], F32, tag="zd", name=f"zd{g}")
                nc.sync.dma_start(z_dram[:], stg[DH : DH + 1, :, :])
                zq = rpool.tile([P, 16], F32, tag="zq", name=f"zq{g}")
                nc.sync.dma_start(zq[:], z_dram[0].rearrange("(p a) -> p a", p=P))
                rq = rpool.tile([P, 16], F32, tag="rq", name=f"rq{g}")
                nc.vector.reciprocal(rq[:], zq[:])
                r_dram = dpool.tile([1, 2 * NT], F32, tag="rd", name=f"rd{g}")
                nc.sync.dma_start(r_dram[0].rearrange("(p a) -> p a", p=P), rq[:])
                for h01 in range(2):
                    rbc = rpool.tile([DH, NT], F32, tag=f"rb{h01}", name=f"rb{g}_{h01}")
                    nc.sync.dma_start(
                        rbc[:],
                        r_dram[0, h01 * NT : (h01 + 1) * NT][None, :].to_broadcast(
                            (DH, NT)
                        ),
                    )
                    for tb in range(2):
                        # early pairs: updk isn't consumed until block 4, so
                        # the 2.3x-slower GpSimd TT is free capacity there;
                        # late pairs stay on DVE because the TT->updk latency
                        # gates the out-proj upd chunks (measured PE stalls
                        # with GpSimd on the late pairs).  No DVE op in this
                        # kernel uses the shared 2-port SBUF modes, so the
                        # GpSimd TTs never block DVE.
                        eng = nc.gpsimd if g <= 2 else nc.vector
                        eng.tensor_tensor(
                            updk[g][h01 * DH : (h01 + 1) * DH, tb * 512 : (tb + 1) * 512],
                            stg[0:DH, 2 * h01 + tb, :],
                            rbc[0:DH, tb * 512 : (tb + 1) * 512],
                            ALU.mult,
                        )

            def av_and_norm(g):
                av_open(g)
                for p4 in range(NSC // 2):
                    av_chunk(g, p4)
                norm(g)

            def out_lhs(kc, mo):
                return wout[:, kc, mo * P : (mo + 1) * P]

            def out_rhs(kc, th):
                sl = slice(th * 512, (th + 1) * 512)
                return tgtTk[kc][:, sl] if kc < NKC else updk[kc - NKC][:, sl]

            up_ps = {}

            def unit_prefix_a(u):
                # out-proj unit, tgt-chunk half: 6 accumulating matmuls.
                # psum stays open for unit_prefix_b (emitted ~2 slots later).
                mo, th = u // 2, u % 2
                ps = pav.tile([P, 512], F32, tag="pav", name=f"op{u}")
                up_ps[u] = ps
                for kc in range(NKC):
                    nc.tensor.matmul(
                        ps[:, :],
                        out_lhs(kc, mo),
                        out_rhs(kc, th),
                        start=(kc == 0),
                        stop=False,
                    )

            def unit_prefix_b(u):
                # upd pairs (0..3 in-stream, 0..4 for tail units), then park
                # the partial in SBUF (bf16) on ACT.
                mo, th = u // 2, u % 2
                last = NKC + 3 if u < 4 else NKC + 4
                ps = up_ps.pop(u)
                for kc in range(NKC, last + 1):
                    nc.tensor.matmul(
                        ps[:, :],
                        out_lhs(kc, mo),
                        out_rhs(kc, th),
                        start=False,
                        stop=(kc == last),
                    )
                if u < 4 and u % 2 == 0:
                    nc.vector.tensor_copy(part_u[u][:], ps[:])
                else:
                    # tail units park on ACT -- it is idle post-stream while
                    # DVE carries the pair-5 normalization chain
                    nc.scalar.copy(part_u[u][:], ps[:])

            def unit_out(u, ps):
                mo, th = u // 2, u % 2
                osb = opool.tile([P, 512], BF16, tag="osb", name=f"osb{u}")
                # alternate the final relu between ACT and DVE so the
                # post-last-matmul trail is halved
                if u % 2 == 0:
                    nc.scalar.activation(osb[:], ps[:], AF.Relu)
                else:
                    nc.vector.tensor_scalar_max(osb[:], ps[:], 0.0)
                q = nc.sync if u % 2 == 0 else nc.scalar
                q.dma_start(
                    outT_d[mo * P : (mo + 1) * P, th * 512 : (th + 1) * 512], osb[:]
                )

            def unit_close(u):
                # remaining upd pairs + identity-matmul fold of the partial.
                mo, th = u // 2, u % 2
                ps = pav.tile([P, 512], F32, tag="pav", name=f"oc{u}")
                first = 10 if u < 4 else 11
                for kc in range(first, 12):
                    nc.tensor.matmul(
                        ps[:, :],
                        out_lhs(kc, mo),
                        out_rhs(kc, th),
                        start=(kc == first),
                        stop=False,
                    )
                nc.tensor.matmul(
                    ps[:, :], ident[:], part_u[u][:], start=False, stop=True
                )
                unit_out(u, ps)

            def unit_straight(u):
                # tail-only: all 12 chunks in one open psum tile -- no
                # park/ident-fold (scores/AV psum pressure is gone by now)
                mo, th = u // 2, u % 2
                ps = pav.tile([P, 512], F32, tag="pav", name=f"os{u}")
                for kc in range(12):
                    nc.tensor.matmul(
                        ps[:, :],
                        out_lhs(kc, mo),
                        out_rhs(kc, th),
                        start=(kc == 0),
                        stop=(kc == 11),
                    )
                unit_out(u, ps)

            # ---- the pipeline: 6 pair-blocks of 8 scores tiles; ACT and DVE
            # drain the two tb halves of the exp stream concurrently; filler
            # PE work (projections, out-proj prefixes) is cut into ~1us
            # chunks emitted one per scores slot; AV bursts sit at sc==3 of
            # the following pair where the previous pair's exps have long
            # drained. ----
            KQ, V, PA, PB = "kq", "v", "pa", "pb"

            def emit(it):
                kind = it[0]
                if kind == KQ:
                    kq_chunk(it[1], it[2], it[3])
                elif kind == V:
                    v_chunk(it[1], it[2])
                elif kind == PA:
                    unit_prefix_a(it[1])
                elif kind == PB:
                    unit_prefix_b(it[1])

            def kq4(g):
                return [(KQ, 0, g, 0), (KQ, 0, g, 1), (KQ, 1, g, 0), (KQ, 1, g, 1)]

            block_fillers = [
                # K(1)/Q(1) first (their W pieces land ~6.5us in); V chunks
                # last (the V columns are the final piece on the sync queue).
                kq4(1) + [(V, 0, 0), (V, 1, 0), (V, 2, 0), (V, 3, 0)],
                [(V, 4, 0), (V, 5, 0), (V, 6, 0), (V, 7, 0)] + kq4(2),
                kq4(3) + [(V, 0, 1), (V, 1, 1), (V, 2, 1), (V, 3, 1)],
                [(V, 4, 1), (V, 5, 1), (V, 6, 1), (V, 7, 1)] + kq4(4),
                kq4(5) + [(PA, 0), (PB, 0), (PA, 1), (PB, 1)],
                [(PA, 2), (PB, 2), (PA, 3), (PB, 3)],
            ]

            for c in range(2):
                kq_chunk(0, 0, c)
            for c in range(2):
                kq_chunk(1, 0, c)

            # GpSimd gate: its first instruction waits on the qTh[0]
            # eviction (~14us), holding all SWDGE HBM traffic (the bulky
            # late-need bf16 tensors) off the wire until the critical
            # startup set has landed.  Emitted AFTER the prologue so Tile
            # sees the RAW dependency.
            nc.gpsimd.tensor_copy(gate[0:1, 0:1], qTh[0][0:1, 0:1])
            for j in range(NKC):
                nc.gpsimd.dma_start(tgtTk[j][:], tgtT_d[j * P : (j + 1) * P, :])
            nc.gpsimd.dma_start(
                wout[:], wout_d[:].rearrange("(k p) n -> p k n", p=P)
            )
            nc.gpsimd.dma_start(ident[:], ident_d[:])

            for g in range(NG):
                scores_open(g)
                fl = list(block_fillers[g])
                for sc in range(NSC):
                    scores_sc(g, sc)
                    # block 0: hold fillers until sc>=2 so the first exps
                    # aren't delayed behind a filler waiting on later DMAs
                    if fl and (g > 0 or sc >= 2):
                        emit(fl.pop(0))
                    if sc == 3 and g >= 1:
                        av_and_norm(g - 1)
                for it in fl:
                    emit(it)

            # tail: pair 5's AV chunks interleave with the out-proj
            # prefixes -- each chunk only needs two more sc of exp(5), so the
            # normalization chain starts right at the last exp instead of
            # after a serial AV burst.
            av_open(NG - 1)
            unit_prefix_a(4)
            av_chunk(NG - 1, 0)
            unit_prefix_b(4)
            av_chunk(NG - 1, 1)
            unit_prefix_a(5)
            av_chunk(NG - 1, 2)
            unit_prefix_b(5)
            av_chunk(NG - 1, 3)
            norm(NG - 1)
            # units 6-9 prefix+park while norm(5)'s ~10us DMA+TT chain is in
            # flight (28 matmuls of updk5-independent work -- measured 5.6us
            # of PE idle when only units 6,7 covered it); closes gate only
            # on updk5 so the relu/DMA trail starts early; units 10,11 run
            # straight through (updk5 long ready, no park/fold needed).
            for u in (6, 7, 8, 9):
                unit_prefix_a(u)
                unit_prefix_b(u)
            for u in range(4):
                unit_close(u)
            unit_close(4)
            unit_close(5)
            unit_straight(10)
            unit_close(6)
            unit_straight(11)
            for u in (7, 8, 9):
                unit_close(u)

    _split_excess_waits(nc)
    return nc


def _make_in_maps(inputs):
    bf = ml_dtypes.bfloat16
    # [P, NKC, cols] with partition p holding row (k*128+p); each weight
    # column group ships as its own contiguous tensor (full-rate DMA)
    w_src8 = np.asarray(inputs["W_src"]).reshape(NKC, P, 2 * OUT).transpose(1, 0, 2)
    w_tgt8 = np.asarray(inputs["W_tgt"]).reshape(NKC, P, OUT).transpose(1, 0, 2)
    wk0 = np.ascontiguousarray(w_src8[:, :, 0:P]).astype(_FP8).reshape(P, NKC * P)
    wkrest = np.ascontiguousarray(w_src8[:, :, P:OUT]).astype(_FP8)
    wval = np.ascontiguousarray(w_src8[:, :, OUT:]).astype(_FP8)
    wq0 = np.ascontiguousarray(w_tgt8[:, :, 0:P]).astype(_FP8).reshape(P, NKC * P)
    wqrest = np.ascontiguousarray(w_tgt8[:, :, P:OUT]).astype(_FP8)
    w_out = np.ascontiguousarray(inputs["W_out"]).astype(bf)
    # biases are structurally zero in this problem -- not shipped to the device
    src = np.asarray(inputs["src"])
    tgt = np.asarray(inputs["tgt"])

    in_maps = []
    for b in range(B):
        # [Ns, SRC] -> [P, NKC, Ns]: partition p holds row (k*128+p) of src^T
        s8 = np.ascontiguousarray(
            src[b].T.reshape(NKC, P, NS).transpose(1, 0, 2)
        ).astype(_FP8)
        t8 = np.ascontiguousarray(
            tgt[b].T.reshape(NKC, P, NT).transpose(1, 0, 2)
        ).astype(_FP8)
        in_maps.append(
            {
                "bsrc": np.ascontiguousarray(
                    np.concatenate([wk0, s8.reshape(P, NKC * NS)], axis=1)
                ),
                "btgt": np.ascontiguousarray(
                    np.concatenate([wq0, t8.reshape(P, NKC * NT)], axis=1)
                ),
                "wkrest": wkrest,
                "wval": wval,
                "wqrest": wqrest,
                "tgtT": np.ascontiguousarray(tgt[b].T).astype(bf),
                "w_out": w_out,
                "ident": _IDENT,
            }
        )
    return in_maps


def kernel(**inputs: np.ndarray) -> np.ndarray:
    global _NC_CACHE
    if _NC_CACHE is None:
        _NC_CACHE = _build_nc()
    nc = _NC_CACHE

    in_maps = _make_in_maps(inputs)
    res = run_bass_kernel_spmd(nc, in_maps, core_ids=list(range(B)))
    return np.stack(
        [np.ascontiguousarray(r["out"].T).astype(np.float32) for r in res.results]
    )


# revision 33
# speedup vs baseline: 1.0156x; 1.0156x over previous
"""Trainium2 Bass kernel for OneSideInterModalityUpdate (dense transformer block).

Reference computation (per batch b, one NeuronCore each -- data-parallel B=8):
    src_tran = relu(src @ W_src)                  [Ns, 2*OUT]
    key, val = split(src_tran)                    [Ns, OUT] each
    q        = relu(tgt @ W_tgt)                  [Nt, OUT]
    per head h (12 heads, DH=64):
        S     = q_h @ k_h^T / sqrt(DH)            [Nt, Ns]
        A     = softmax(S, axis=-1)
        upd_h = A @ v_h                           [Nt, DH]
    out = relu([tgt, upd] @ W_out)                [Nt, OUT]

Performance design (baseline measured: PE 175us busy of a 218us span,
ACT 160us busy -- dual-bottleneck.  This version attacks both; measured
~162us at the fast chip P-state, ~124us PE busy, both exp engines
~100us):

  - K/Q/V projections run fp8e4 DoubleRow (src/tgt/W_src/W_tgt shipped
    as fp8): contraction 768 = 3 dual-128 k-tile instructions instead of
    6, nearly halving projection PE streaming (fp8 dot noise is ~5% per
    K/Q/V element but attention-averages out over ~800 source positions;
    numpy-validated end-to-end at 2.8e-3 vs 2.5e-3 for bf16).  Scores,
    AV (fp8 es/v65, DoubleRow) and the out-projection (bf16) keep the
    bf16 layouts and numerics.
  - The exp stream -- formerly a ~129us serial ACT chain -- is split
    across TWO engines: one tb-half of each scores tile exps on ACT
    (exact, scale/bias folded); the other half is computed on the DVE as
    a ONE-instruction Schraudolph: fp8e4 bits of exp(x) are
    round(x*8/ln2 + 56), i.e. one tensor_scalar(mult,add) f32->int8
    written through an int8 bitcast of the fp8 es tile.  The fp8 grid
    (ulp = 2^(1/8)) swallows the mantissa-linearization error, so this
    is exactly as accurate as exact-exp-then-cast-to-fp8.  The
    ACT/DVE assignment alternates per source chunk so consecutive frees
    of each scores-psum ring slot come from both engines (halves
    ring-wait stalls on the PE).
  - V gets a ones-column (65th lhsT column) so AV psum row 64 is the
    softmax denominator Z for free; Z -> DRAM -> [128,16] repack ->
    reciprocal -> broadcast-back normalization.  The early pairs'
    normalization multiplies ride the otherwise-idle GpSimd.
  - Output projection TRANSPOSED (wout stationary), prefix/park/close
    split; after the last exp, units 6-9 prefix+park (28 matmuls)
    covers the pair-5 normalization chain's ~10us latency before
    anything gates on updk5.  Both t-halves of an output row-block
    ship as one [128,1024] DMA.  Host transposes the [OUT, NT] bf16
    result back.
  - Startup: each HWDGE queue's FIRST transfer is one contiguous blob
    carrying everything its side needs (W column-block 0 + all six
    activation chunk-pairs) -- per-dma_start completion latency (~2us)
    serializes per queue, so fewer, larger, contiguous transfers put
    the first matmul at ~12.5us (framework preamble is ~7us of that)
    vs ~22us for the baseline.  The bulky late-need bf16 tensors ride
    SWDGE gated behind an early eviction so they can't steal HBM
    bandwidth from the critical set.
"""

import numpy as np
import ml_dtypes

import concourse.bass as bass
import concourse.mybir as mybir
import concourse.tile as tile
from concourse import library_config
from concourse.bass_utils import run_bass_kernel_spmd

BF16 = mybir.dt.bfloat16
F32 = mybir.dt.float32
FP8 = mybir.dt.float8e4
I8 = mybir.dt.int8
AF = mybir.ActivationFunctionType
ALU = mybir.AluOpType
PM = mybir.MatmulPerfMode

B, NS, NT = 8, 1024, 1024
SRC, TGT, OUT, H = 768, 768, 768, 12
_IDENT = np.eye(128, dtype=ml_dtypes.bfloat16)
_FP8 = ml_dtypes.float8_e4m3
DH = OUT // H            # 64
P = 128
NKC = SRC // P           # 6 contraction chunks for the projections
NJ = NKC // 2            # 3 dual-k-tile DoubleRow steps
NSC = NS // P            # 8 source chunks
NG = H // 2              # 6 head pairs
NU = 12                  # out-proj units: 6 o-chunks x 2 t-halves
SCALE = 1.0 / np.sqrt(DH)
# Schraudolph exp->fp8e4 bits: bits = round((s*SCALE - 1) * 8/ln2 + 56)
_L2E8 = 8.0 / np.log(2.0)
EXPA = float(SCALE * _L2E8)
EXPB = float(56.0 - _L2E8)

_NC_CACHE = None


def _split_excess_waits(nc, keep=1):
    """This container's walrus encodes at most ONE sync-wait per instruction,
    but the Tile scheduler can attach several (notably on the final drain).
    Split excess waits onto preceding same-engine NoOp carriers."""
    for fn in nc.m.functions:
        for bb in fn.blocks:
            il = list(bb.instructions)
            out = []
            changed = False
            for inst in il:
                si = inst.sync_info
                if si is not None and len(si.on_wait) > keep:
                    waits = list(si.on_wait)
                    changed = True
                    ncarry = len(waits) - keep
                    for i0 in range(0, ncarry, keep):
                        nop = mybir.InstNoOp(
                            name=nc.get_next_instruction_name(),
                            opcode="NoOp",
                            engine=inst.engine,
                            debug=inst.debug,
                            ins=[],
                            outs=[],
                            descendants=None,
                            sync_info=mybir.SyncInfo(
                                on_wait=waits[i0 : i0 + keep], on_update=[]
                            ),
                            bass_sim_breakpoint=False,
                            bass_priority=None,
                            bass_wait_until_ts=None,
                            bass_scheduled_tick=None,
                            bass_scheduled_proc=None,
                            bass_scheduled_scope=None,
                            bass_addl_debug=None,
                            text_hint="wait_carrier",
                            bass_nofuse=True,
                        )
                        nc.register_instruction(nop)
                        out.append(nop)
                    inst.sync_info = mybir.SyncInfo(
                        on_wait=waits[ncarry:], on_update=list(si.on_update)
                    )
                out.append(inst)
            if changed:
                bb.instructions = out
    return nc


def _build_nc() -> bass.Bass:
    nc = bass.Bass()

    # Each HWDGE queue's FIRST transfer is the whole critical startup set
    # for its side, packed contiguously: every dma_start pays ~2us of fixed
    # completion latency that serializes on its queue, so two 4-piece queues
    # put the first matmul at ~13.5us -- one blob per queue puts it at ~12.5
    # with the full set landed.  blob = [W column-block 0 | all 6 activation
    # chunk-pairs] per partition.
    BLOB = NKC * P + NKC * NS  # 6912 fp8 bytes per partition
    bsrc_d = nc.dram_tensor("bsrc", [P, BLOB], FP8, kind="ExternalInput")
    btgt_d = nc.dram_tensor("btgt", [P, BLOB], FP8, kind="ExternalInput")
    # remaining W pieces ship as separate contiguous tensors: a column-slice
    # DMA out of one big [P, k, 1536] tensor moves 128-640B elements and
    # crawls at ~60GB/s on the wire (measured: the K-rest slice completed at
    # 31us and stalled the PE 9us); per-piece tensors stream at full rate.
    wkrest_d = nc.dram_tensor("wkrest", [P, NKC, OUT - P], FP8, kind="ExternalInput")
    wval_d = nc.dram_tensor("wval", [P, NKC, OUT], FP8, kind="ExternalInput")
    wqrest_d = nc.dram_tensor("wqrest", [P, NKC, OUT - P], FP8, kind="ExternalInput")
    tgtT_d = nc.dram_tensor("tgtT", [TGT, NT], BF16, kind="ExternalInput")
    wout_d = nc.dram_tensor("w_out", [OUT + TGT, OUT], BF16, kind="ExternalInput")
    outT_d = nc.dram_tensor("out", [OUT, NT], BF16, kind="ExternalOutput")
    ident_d = nc.dram_tensor("ident", [P, P], BF16, kind="ExternalInput")

    with tile.TileContext(nc) as tc:
        with (
            tc.tile_pool(name="const", bufs=1) as cpool,
            tc.tile_pool(name="es", bufs=2) as epool,
            tc.tile_pool(name="rr", bufs=2) as rpool,
            tc.tile_pool(name="outsb", bufs=3) as opool,
            tc.tile_pool(name="pss", bufs=2, space="PSUM") as pss,
            tc.tile_pool(name="pav", bufs=4, space="PSUM") as pav,
            tc.tile_pool(name="dram", bufs=2, space="DRAM") as dpool,
        ):
            # ---- persistent SBUF tensors ----
            bsrc = cpool.tile([P, BLOB], FP8)
            btgt = cpool.tile([P, BLOB], FP8)
            wk0 = bsrc[:, 0 : NKC * P].rearrange("p (k c) -> p k c", k=NKC)
            src8 = bsrc[:, NKC * P :].rearrange("p (k n) -> p k n", k=NKC)
            wq0 = btgt[:, 0 : NKC * P].rearrange("p (k c) -> p k c", k=NKC)
            tgt8 = btgt[:, NKC * P :].rearrange("p (k n) -> p k n", k=NKC)
            wkrest = cpool.tile([P, NKC, OUT - P], FP8)
            wval = cpool.tile([P, NKC, OUT], FP8)
            wqrest = cpool.tile([P, NKC, OUT - P], FP8)
            tgtTk = [cpool.tile([P, NT], BF16, name=f"tgtc{j}") for j in range(NKC)]
            wout = cpool.tile([P, 2 * NKC, OUT], BF16)
            kTh = [cpool.tile([P, NS], BF16, name=f"kT{g}") for g in range(NG)]
            qTh = [cpool.tile([P, NT], BF16, name=f"qT{g}") for g in range(NG)]
            # [s, h, sc, dh+ones+pad]: the sc slot is padded to 80 bytes because
            # DoubleRow LDWEIGHTS requires the dual-k-tile stride %16 == 0.
            v65 = cpool.tile([P, H, NSC, 80], FP8)
            updk = [cpool.tile([P, NT], BF16, name=f"upd{g}") for g in range(NG)]
            part_u = [cpool.tile([P, 512], BF16, name=f"pout{u}") for u in range(NU)]
            ident = cpool.tile([P, P], BF16)
            neg1 = cpool.tile([P, 1], F32)
            prime = cpool.tile([1, 1], F32)

            # ---- input DMAs.  The two HWDGE queues carry everything the
            # stream needs, strictly in consumption order -- queue order
            # serializes the transfers, so the non-critical pieces cannot
            # steal HBM bandwidth from the first-needed ones.  The bulky
            # bf16 late-need tensors (tgtT chunks for the out-proj moving
            # side, wout, ident; needed only from block 4, ~80us in) ride
            # SWDGE on the Pool engine, which is GATED below so its HBM
            # pulls don't overlap the critical startup set either. ----
            nc.sync.dma_start(bsrc[:], bsrc_d[:])
            nc.scalar.dma_start(btgt[:], btgt_d[:])
            nc.sync.dma_start(wkrest[:], wkrest_d[:])
            nc.scalar.dma_start(wqrest[:], wqrest_d[:])
            nc.sync.dma_start(wval[:], wval_d[:])
            gate = cpool.tile([1, 1], BF16)

            nc.vector.memset(v65[:, :, :, DH], 1.0)  # ones column for Z
            nc.vector.memset(neg1[:], -1.0)  # exp bias (cancels in softmax)
            # prime the ACT exp table NOW -- the implicit load otherwise
            # rides along with the first real exp's dispatch (1.3us late)
            nc.scalar.activation(prime[:], neg1[0:1, :], AF.Exp)

            # ---- building blocks ----
            nev = [0]

            def kq_chunk(which, mo, tb):
                # one [128,512] half of a K^T/Q^T projection column block:
                # 3 accumulating fp8 DoubleRow matmuls + relu evict
                # (alternating DVE/ACT -- both carry half the exp stream).
                w0, wrest, act8, dst = (
                    (wk0, wkrest, src8, kTh)
                    if which == 0
                    else (wq0, wqrest, tgt8, qTh)
                )
                ps = pav.tile([P, 512], F32, tag="pav", name=f"kq{which}_{mo}_{tb}")
                sl = slice(tb * 512, (tb + 1) * 512)
                for j in range(NJ):
                    w = (
                        w0[:, 2 * j : 2 * j + 2, :]
                        if mo == 0
                        else wrest[:, 2 * j : 2 * j + 2, (mo - 1) * P : mo * P]
                    )
                    nc.tensor.matmul(
                        ps[:],
                        w,
                        act8[:, 2 * j : 2 * j + 2, sl],
                        start=(j == 0),
                        stop=(j == NJ - 1),
                        perf_mode=PM.DoubleRow,
                    )
                nev[0] += 1
                if nev[0] % 2 == 0:
                    nc.vector.tensor_scalar_max(dst[mo][:, sl], ps[:], 0.0)
                else:
                    nc.scalar.activation(dst[mo][:, sl], ps[:], AF.Relu)

            es_tiles = {}

            def scores_open(g):
                es_tiles[g] = (
                    epool.tile([P, 2, NSC, 512], FP8, tag="esA", name=f"esA{g}"),
                    epool.tile([P, 2, NSC, 512], FP8, tag="esB", name=f"esB{g}"),
                )

            def scores_sc(g, sc):
                esA, esB = es_tiles[g]
                for tb, es in ((0, esA), (1, esB)):
                    ps = pss.tile([P, 2, 512], F32, tag="pss", name=f"sc{g}_{sc}")
                    for h01 in range(2):
                        hp = h01 * DH
                        nc.tensor.matmul(
                            ps[:, h01, :],
                            kTh[g][hp : hp + DH, sc * P : (sc + 1) * P],
                            qTh[g][hp : hp + DH, tb * 512 : (tb + 1) * 512],
                            start=True,
                            stop=True,
                        )
                    # the psum-ring slot for tb is freed by whichever engine
                    # exps this tile; alternating the assignment per sc means
                    # consecutive frees of a slot come from BOTH engines, so
                    # one engine running momentarily behind doesn't stall the
                    # next scores matmul (measured ~0.7us stalls otherwise).
                    if tb == (sc % 2):
                        # bias -1 (cancels in softmax) keeps es in [0.4, 110]:
                        # comfortably inside fp8e4's finite range, far above
                        # its subnormal floor.
                        nc.scalar.activation(
                            es[:, :, sc], ps[:], AF.Exp, bias=neg1[:], scale=SCALE
                        )
                    else:
                        # Schraudolph: fp8 bits of exp(s*SCALE-1) in ONE DVE op
                        nc.vector.tensor_scalar(
                            es[:, :, sc].bitcast(I8),
                            ps[:],
                            EXPA,
                            EXPB,
                            ALU.mult,
                            ALU.add,
                        )

            def v_chunk(sc, vh):
                # V columns for head-half vh (6 heads), one source chunk,
                # via 3 DoubleRow matmuls (src chunk stationary, W_val moving)
                o0 = vh * 384
                ps = pav.tile([P, 384], F32, tag="pav", name=f"vp{vh}_{sc}")
                for j in range(NJ):
                    nc.tensor.matmul(
                        ps[:, :],
                        src8[:, 2 * j : 2 * j + 2, sc * P : (sc + 1) * P],
                        wval[:, 2 * j : 2 * j + 2, o0 : o0 + 384],
                        start=(j == 0),
                        stop=(j == NJ - 1),
                        perf_mode=PM.DoubleRow,
                    )
                nev[0] += 1
                dst = v65[:, 6 * vh : 6 * (vh + 1), sc, 0:DH]
                srcv = ps[:].rearrange("p (h c) -> p h c", c=DH)
                if nev[0] % 2 == 0:
                    nc.vector.tensor_scalar_max(dst, srcv, 0.0)
                else:
                    nc.scalar.activation(dst, srcv, AF.Relu)

            pu_live = {}

            def av_open(g):
                pu_live[g] = [
                    [
                        pav.tile([P, 512], F32, tag="pav", name=f"pu{g}_{h01}_{tb}")
                        for tb in range(2)
                    ]
                    for h01 in range(2)
                ]

            def av_chunk(g, p4):
                esA, esB = es_tiles[g]
                pu = pu_live[g]
                for h01 in range(2):
                    h = 2 * g + h01
                    for tb, es in ((0, esA), (1, esB)):
                        nc.tensor.matmul(
                            pu[h01][tb][0 : DH + 1, :],
                            v65[:, h, 2 * p4 : 2 * p4 + 2, 0 : DH + 1],
                            es[:, h01, 2 * p4 : 2 * p4 + 2, :],
                            start=(p4 == 0),
                            stop=(p4 == NSC // 2 - 1),
                            perf_mode=PM.DoubleRow,
                        )

            def norm(g):
                # evict rows 0..64 of each psum quarter into one staging
                # mega-tile (frees the pav slots fast -- the ring is shared
                # with the projection/out-proj chunks), then: Z row -> DRAM
                # -> [128,16] repack -> cheap reciprocal -> DRAM -> broadcast
                # R -> normalize into updk.
                es_tiles.pop(g)
                pu = pu_live.pop(g)
                stg = rpool.tile([P, 4, 512], F32, tag="stg", name=f"stg{g}")
                for h01 in range(2):
                    for tb in range(2):
                        # last pair: the exp stream is over and this chain
                        # gates the final closes -- split across both engines
                        eng = (
                            (nc.scalar if tb == 0 else nc.vector)
                            if g == NG - 1
                            else nc.scalar
                        )
                        if eng is nc.vector:
                            eng.tensor_copy(
                                stg[0 : DH + 1, 2 * h01 + tb, :],
                                pu[h01][tb][0 : DH + 1, :],
                            )
                        else:
                            eng.copy(
                                stg[0 : DH + 1, 2 * h01 + tb, :],
                                pu[h01][tb][0 : DH + 1, :],
                            )
                z_dram = dpool.tile([1, 2 * NT], F32, tag="zd", name=f"zd{g}")
                nc.sync.dma_start(z_dram[:], stg[DH : DH + 1, :, :])
                zq = rpool.tile([P, 16], F32, tag="zq", name=f"zq{g}")
                nc.sync.dma_start(zq[:], z_dram[0].rearrange("(p a) -> p a", p=P))
                rq = rpool.tile([P, 16], F32, tag="rq", name=f"rq{g}")
                nc.vector.reciprocal(rq[:], zq[:])
                r_dram = dpool.tile([1, 2 * NT], F32, tag="rd", name=f"rd{g}")
                nc.sync.dma_start(r_dram[0].rearrange("(p a) -> p a", p=P), rq[:])
                for h01 in range(2):
                    rbc = rpool.tile([DH, NT], F32, tag=f"rb{h01}", name=f"rb{g}_{h01}")
                    nc.sync.dma_start(
                        rbc[:],
                        r_dram[0, h01 * NT : (h01 + 1) * NT][None, :].to_broadcast(
                            (DH, NT)
                        ),
                    )
                    for tb in range(2):
                        # early pairs: updk isn't consumed until block 4, so
                        # the 2.3x-slower GpSimd TT is free capacity there;
                        # late pairs stay on DVE because the TT->updk latency
                        # gates the out-proj upd chunks (measured PE stalls
                        # with GpSimd on the late pairs).  No DVE op in this
                        # kernel uses the shared 2-port SBUF modes, so the
                        # GpSimd TTs never block DVE.
                        eng = nc.gpsimd if g <= 2 else nc.vector
                        eng.tensor_tensor(
                            updk[g][h01 * DH : (h01 + 1) * DH, tb * 512 : (tb + 1) * 512],
                            stg[0:DH, 2 * h01 + tb, :],
                            rbc[0:DH, tb * 512 : (tb + 1) * 512],
                            ALU.mult,
                        )

            def av_and_norm(g):
                av_open(g)
                for p4 in range(NSC // 2):
                    av_chunk(g, p4)
                norm(g)

            def out_lhs(kc, mo):
                return wout[:, kc, mo * P : (mo + 1) * P]

            def out_rhs(kc, th):
                sl = slice(th * 512, (th + 1) * 512)
                return tgtTk[kc][:, sl] if kc < NKC else updk[kc - NKC][:, sl]

            up_ps = {}

            def unit_prefix_a(u):
                # out-proj unit, tgt-chunk half: 6 accumulating matmuls.
                # psum stays open for unit_prefix_b (emitted ~2 slots later).
                mo, th = u // 2, u % 2
                ps = pav.tile([P, 512], F32, tag="pav", name=f"op{u}")
                up_ps[u] = ps
                for kc in range(NKC):
                    nc.tensor.matmul(
                        ps[:, :],
                        out_lhs(kc, mo),
                        out_rhs(kc, th),
                        start=(kc == 0),
                        stop=False,
                    )

            def unit_prefix_b(u):
                # upd pairs (0..3 in-stream, 0..4 for tail units), then park
                # the partial in SBUF (bf16) on ACT.
                mo, th = u // 2, u % 2
                last = NKC + 3 if u < 4 else NKC + 4
                ps = up_ps.pop(u)
                for kc in range(NKC, last + 1):
                    nc.tensor.matmul(
                        ps[:, :],
                        out_lhs(kc, mo),
                        out_rhs(kc, th),
                        start=False,
                        stop=(kc == last),
                    )
                if u < 4 and u % 2 == 0:
                    nc.vector.tensor_copy(part_u[u][:], ps[:])
                else:
                    # tail units park on ACT -- it is idle post-stream while
                    # DVE carries the pair-5 normalization chain
                    nc.scalar.copy(part_u[u][:], ps[:])

            osb_live = {}

            def unit_out(u, ps):
                # both t-halves of an output row-block collect into one
                # [P,1024] tile and ship as ONE DMA: 6 output DMAs instead
                # of 12 halves the end-of-kernel queue-issue serialization.
                mo, th = u // 2, u % 2
                if mo in osb_live:
                    osb = osb_live.pop(mo)
                    done = True
                else:
                    osb = opool.tile([P, NT], BF16, tag="osb", name=f"osb{mo}")
                    osb_live[mo] = osb
                    done = False
                sl = slice(th * 512, (th + 1) * 512)
                # alternate the final relu between ACT and DVE so the
                # post-last-matmul trail is halved
                if u % 2 == 0:
                    nc.scalar.activation(osb[:, sl], ps[:], AF.Relu)
                else:
                    nc.vector.tensor_scalar_max(osb[:, sl], ps[:], 0.0)
                if done:
                    q = nc.sync if mo % 2 == 0 else nc.scalar
                    q.dma_start(outT_d[mo * P : (mo + 1) * P, :], osb[:])

            def unit_close(u):
                # remaining upd pairs + identity-matmul fold of the partial.
                mo, th = u // 2, u % 2
                ps = pav.tile([P, 512], F32, tag="pav", name=f"oc{u}")
                first = 10 if u < 4 else 11
                for kc in range(first, 12):
                    nc.tensor.matmul(
                        ps[:, :],
                        out_lhs(kc, mo),
                        out_rhs(kc, th),
                        start=(kc == first),
                        stop=False,
                    )
                nc.tensor.matmul(
                    ps[:, :], ident[:], part_u[u][:], start=False, stop=True
                )
                unit_out(u, ps)

            def unit_straight(u):
                # tail-only: all 12 chunks in one open psum tile -- no
                # park/ident-fold (scores/AV psum pressure is gone by now)
                mo, th = u // 2, u % 2
                ps = pav.tile([P, 512], F32, tag="pav", name=f"os{u}")
                for kc in range(12):
                    nc.tensor.matmul(
                        ps[:, :],
                        out_lhs(kc, mo),
                        out_rhs(kc, th),
                        start=(kc == 0),
                        stop=(kc == 11),
                    )
                unit_out(u, ps)

            # ---- the pipeline: 6 pair-blocks of 8 scores tiles; ACT and DVE
            # drain the two tb halves of the exp stream concurrently; filler
            # PE work (projections, out-proj prefixes) is cut into ~1us
            # chunks emitted one per scores slot; AV bursts sit at sc==3 of
            # the following pair where the previous pair's exps have long
            # drained. ----
            KQ, V, PA, PB = "kq", "v", "pa", "pb"

            def emit(it):
                kind = it[0]
                if kind == KQ:
                    kq_chunk(it[1], it[2], it[3])
                elif kind == V:
                    v_chunk(it[1], it[2])
                elif kind == PA:
                    unit_prefix_a(it[1])
                elif kind == PB:
                    unit_prefix_b(it[1])

            def kq4(g):
                return [(KQ, 0, g, 0), (KQ, 0, g, 1), (KQ, 1, g, 0), (KQ, 1, g, 1)]

            block_fillers = [
                # K(1)/Q(1) first (their W pieces land ~6.5us in); V chunks
                # last (the V columns are the final piece on the sync queue).
                kq4(1) + [(V, 0, 0), (V, 1, 0), (V, 2, 0), (V, 3, 0)],
                [(V, 4, 0), (V, 5, 0), (V, 6, 0), (V, 7, 0)] + kq4(2),
                kq4(3) + [(V, 0, 1), (V, 1, 1), (V, 2, 1), (V, 3, 1)],
                [(V, 4, 1), (V, 5, 1), (V, 6, 1), (V, 7, 1)] + kq4(4),
                kq4(5) + [(PA, 0), (PB, 0), (PA, 1), (PB, 1)],
                [(PA, 2), (PB, 2), (PA, 3), (PB, 3)],
            ]

            for c in range(2):
                kq_chunk(0, 0, c)
            for c in range(2):
                kq_chunk(1, 0, c)

            # GpSimd gate: its first instruction waits on the qTh[0]
            # eviction (~14us), holding all SWDGE HBM traffic (the bulky
            # late-need bf16 tensors) off the wire until the critical
            # startup set has landed.  Emitted AFTER the prologue so Tile
            # sees the RAW dependency.
            nc.gpsimd.tensor_copy(gate[0:1, 0:1], qTh[0][0:1, 0:1])
            for j in range(NKC):
                nc.gpsimd.dma_start(tgtTk[j][:], tgtT_d[j * P : (j + 1) * P, :])
            nc.gpsimd.dma_start(
                wout[:], wout_d[:].rearrange("(k p) n -> p k n", p=P)
            )
            nc.gpsimd.dma_start(ident[:], ident_d[:])

            for g in range(NG):
                scores_open(g)
                fl = list(block_fillers[g])
                for sc in range(NSC):
                    scores_sc(g, sc)
                    # block 0: hold fillers until sc>=2 so the first exps
                    # aren't delayed behind a filler waiting on later DMAs
                    if fl and (g > 0 or sc >= 2):
                        emit(fl.pop(0))
                    if sc == 3 and g >= 1:
                        av_and_norm(g - 1)
                for it in fl:
                    emit(it)

            # tail: pair 5's AV chunks interleave with the out-proj
            # prefixes -- each chunk only needs two more sc of exp(5), so the
            # normalization chain starts right at the last exp instead of
            # after a serial AV burst.
            av_open(NG - 1)
            unit_prefix_a(4)
            av_chunk(NG - 1, 0)
            unit_prefix_b(4)
            av_chunk(NG - 1, 1)
            unit_prefix_a(5)
            av_chunk(NG - 1, 2)
            unit_prefix_b(5)
            av_chunk(NG - 1, 3)
            norm(NG - 1)
            # units 6-9 prefix+park while norm(5)'s ~10us DMA+TT chain is in
            # flight (28 matmuls of updk5-independent work -- measured 5.6us
            # of PE idle when only units 6,7 covered it); closes gate only
            # on updk5 so the relu/DMA trail starts early; units 10,11 run
            # straight through (updk5 long ready, no park/fold needed).
            for u in (6, 7, 8, 9):
                unit_prefix_a(u)
                unit_prefix_b(u)
            for u in range(4):
                unit_close(u)
            unit_close(4)
            unit_close(5)
            unit_straight(10)
            unit_close(6)
            unit_straight(11)
            for u in (7, 8, 9):
                unit_close(u)

    _split_excess_waits(nc)
    return nc


def _make_in_maps(inputs):
    bf = ml_dtypes.bfloat16
    # [P, NKC, cols] with partition p holding row (k*128+p); each weight
    # column group ships as its own contiguous tensor (full-rate DMA)
    w_src8 = np.asarray(inputs["W_src"]).reshape(NKC, P, 2 * OUT).transpose(1, 0, 2)
    w_tgt8 = np.asarray(inputs["W_tgt"]).reshape(NKC, P, OUT).transpose(1, 0, 2)
    wk0 = np.ascontiguousarray(w_src8[:, :, 0:P]).astype(_FP8).reshape(P, NKC * P)
    wkrest = np.ascontiguousarray(w_src8[:, :, P:OUT]).astype(_FP8)
    wval = np.ascontiguousarray(w_src8[:, :, OUT:]).astype(_FP8)
    wq0 = np.ascontiguousarray(w_tgt8[:, :, 0:P]).astype(_FP8).reshape(P, NKC * P)
    wqrest = np.ascontiguousarray(w_tgt8[:, :, P:OUT]).astype(_FP8)
    w_out = np.ascontiguousarray(inputs["W_out"]).astype(bf)
    # biases are structurally zero in this problem -- not shipped to the device
    src = np.asarray(inputs["src"])
    tgt = np.asarray(inputs["tgt"])

    in_maps = []
    for b in range(B):
        # [Ns, SRC] -> [P, NKC, Ns]: partition p holds row (k*128+p) of src^T
        s8 = np.ascontiguousarray(
            src[b].T.reshape(NKC, P, NS).transpose(1, 0, 2)
        ).astype(_FP8)
        t8 = np.ascontiguousarray(
            tgt[b].T.reshape(NKC, P, NT).transpose(1, 0, 2)
        ).astype(_FP8)
        in_maps.append(
            {
                "bsrc": np.ascontiguousarray(
                    np.concatenate([wk0, s8.reshape(P, NKC * NS)], axis=1)
                ),
                "btgt": np.ascontiguousarray(
                    np.concatenate([wq0, t8.reshape(P, NKC * NT)], axis=1)
                ),
                "wkrest": wkrest,
                "wval": wval,
                "wqrest": wqrest,
                "tgtT": np.ascontiguousarray(tgt[b].T).astype(bf),
                "w_out": w_out,
                "ident": _IDENT,
            }
        )
    return in_maps


def kernel(**inputs: np.ndarray) -> np.ndarray:
    global _NC_CACHE
    if _NC_CACHE is None:
        _NC_CACHE = _build_nc()
    nc = _NC_CACHE

    in_maps = _make_in_maps(inputs)
    res = run_bass_kernel_spmd(nc, in_maps, core_ids=list(range(B)))
    return np.stack(
        [np.ascontiguousarray(r["out"].T).astype(np.float32) for r in res.results]
    )
